# revision 47
# baseline (speedup 1.0000x reference)
"""Chamfer distance loss on Trainium2 (Bass/Tile), 8-core SPMD.

Reference math per batch b (inp/tgt: (B, C, N), mask: (B, N)):
    x = inp[b].T * mask[b,:,None]   # (N, 3)
    y = tgt[b].T * mask[b,:,None]
    d[n,m]  = ||x_n||^2 + ||y_m||^2 - 2 x_n.y_m
    loss    = mean(min_m d) + mean(min_n d)      (means over all B*N)

Device decomposition (data-parallel, 2 batches per core), using linearity
of the mean to split the norm terms out of the min:
    sum_n dist1 = sum_n min_m (y2[m] - 2 x_n.y_m) + sum_n x2[n]
    sum_m dist2 = sum_m min_n (x2[n] - 2 x_n.y_m) + sum_m y2[m]
Each min_m pass is a K=4 augmented matmul  [x0,x1,x2,1]^T . [-2y0,-2y1,-2y2,y2]
producing g[n,m] rows in PSUM, reduced with row-min only (no partition
reductions anywhere).  Row mins use the fused DVE tensor_tensor_reduce
(min elementwise of a PSUM half and an ACT-copied SBUF half, then min
along the free axis), so DVE and ACT split the reduction bandwidth.
Matmuls run as float32r (full PE rate at free dim 512, vs 4x slower fp32).

Host: shard batches across 8 cores, run SPMD, sum the per-core partial
sums and divide by B*N.
"""

import numpy as np

B, C, N = 16, 3, 4096
NCORES = 8
BPC = B // NCORES        # batches per core
NT = N // 128            # 32 n-tiles per pass
HALF = N // 2            # 2048 = one 4-bank PSUM group
BIG = float(np.finfo(np.float32).max)

_CACHE = {}


def _build():
    """Build the single-core Bass program (same program runs on all 8 cores
    with different input data)."""
    from contextlib import ExitStack

    from concourse import bacc, bass, mybir, tile  # noqa: F401

    f32 = mybir.dt.float32
    f32r = mybir.dt.float32r
    Alu = mybir.AluOpType

    nc = bacc.Bacc(trn_type="TRN2", target_bir_lowering=False, debug=False)

    inp_d = nc.dram_tensor("inp", [BPC, C, N], f32, kind="ExternalInput").ap()
    tgt_d = nc.dram_tensor("tgt", [BPC, C, N], f32, kind="ExternalInput").ap()
    mask_d = nc.dram_tensor("mask", [BPC, N], f32, kind="ExternalInput").ap()
    # Per-partition partial sums; host sums all of them and divides by B*N.
    out_d = nc.dram_tensor("out", [128, 1], f32, kind="ExternalOutput").ap()

    with tile.TileContext(nc) as tc, ExitStack() as ctx:
        pool = ctx.enter_context(tc.tile_pool(name="main", bufs=1))

        # Every compute op's partition pattern must start in row group 0
        # (offset < stride) or at 32/64/96, so operands live directly in
        # row-group-aligned mega-tiles, pass pb at partitions 32pb..32pb+3:
        #   wt group pb: rows 0-2 = -2*masked data, row 3 = 1.0   (stationary)
        #   rt group pb: rows 0-2 =    masked data, row 3 = norm  (moving)
        # Pass pairing (lhsT.T @ rhs = norm[m] - 2 v.w):
        #   pb0: -2x(b0) . y(b0)   pb1: -2y(b0) . x(b0)   pb2/pb3: batch 1
        # Final operands are float32r typed: the rounding copy below is their
        # sole writer, which satisfies the BIR verifier's "rounded to FP32r"
        # requirement for fp32r matmul inputs.
        wtr = pool.tile([128, N], f32r)
        rtr = pool.tile([128, N], f32r)
        # Columns 0..127: row-min of each (pass, n-tile)'s first PSUM half;
        # columns 128..255: second half; column 256: norm sums (rows 0-3).
        mincols = pool.tile([128, 8 * NT + 1], f32)
        fmin = pool.tile([128, 4 * NT], f32)
        r1a = pool.tile([128, 1], f32)
        r1 = pool.tile([128, 1], f32)

        nc.gpsimd.memset(mincols[:], 0.0)

        with tc.tile_pool(name="prep", bufs=1) as prpool:
            wt = prpool.tile([128, N], f32)
            rt = prpool.tile([128, N], f32)
            mr = prpool.tile([128, N], f32)  # per-batch mask rows (data rows only)
            dd = prpool.tile([4, 3 * N], f32)  # group pb's 3 masked rows, concat
            n4 = prpool.tile([4, N], f32)    # norm rows, group-major
            ones4 = prpool.tile([4, N], f32)

            nc.gpsimd.memset(ones4[:], 1.0)

            # Raw loads; group sources: wt <- (x0,y0,x1,y1), rt <- (y0,x0,y1,x1)
            # Only rows the matmuls read (32pb+0..3) are ever written/read, so
            # wt/rt need no zero-fill and every DMA below has at most 1 wait
            # (the DMA descriptor path supports only a single sync wait).
            for pb, b in enumerate((0, 0, 1, 1)):
                wsrc = inp_d[b] if pb % 2 == 0 else tgt_d[b]
                rsrc = tgt_d[b] if pb % 2 == 0 else inp_d[b]
                nc.gpsimd.dma_start(out=wt[32 * pb : 32 * pb + 3, :], in_=wsrc)
                nc.gpsimd.dma_start(out=rt[32 * pb : 32 * pb + 3, :], in_=rsrc)
                # DRAM-source broadcast AP: one DMA replicates the mask row.
                nc.gpsimd.dma_start(
                    out=mr[32 * pb : 32 * pb + 3, :],
                    in_=mask_d[b : b + 1, :].broadcast_to((3, N)),
                )

            # Mask data rows; -2 scale on wt's data rows; ones rows via DMA.
            for pb in range(4):
                g = slice(32 * pb, 32 * pb + 3)
                nc.vector.tensor_mul(rt[g, :], rt[g, :], mr[g, :])
                nc.vector.tensor_mul(wt[g, :], wt[g, :], mr[g, :])
                nc.scalar.mul(wt[g, :], wt[g, :], -2.0)
                nc.gpsimd.dma_start(
                    out=wt[32 * pb + 3 : 32 * pb + 4, :], in_=ones4[pb : pb + 1, :]
                )

            # Norm rows: gather each group's 3 masked rows into partition pb
            # (concatenated along free dim), square in place, sum the spans.
            for pb in range(4):
                nc.gpsimd.dma_start(
                    out=dd[pb : pb + 1, :], in_=rt[32 * pb : 32 * pb + 3, :]
                )
            nc.vector.tensor_mul(dd[:], dd[:], dd[:])
            nc.vector.tensor_add(n4[:], dd[:, 0:N], dd[:, N : 2 * N])
            nc.vector.tensor_add(n4[:], n4[:], dd[:, 2 * N : 3 * N])
            for pb in range(4):
                nc.gpsimd.dma_start(
                    out=rt[32 * pb + 3 : 32 * pb + 4, :], in_=n4[pb : pb + 1, :]
                )
            # sum_n x2 + sum_m y2 terms (linearity of the mean) go straight
            # into mincols' extra column.
            nc.vector.tensor_reduce(
                mincols[0:4, 8 * NT : 8 * NT + 1],
                n4[:],
                axis=mybir.AxisListType.X,
                op=Alu.add,
            )

            # Rounding copies into the fp32r operand tiles, per group (the
            # in-between rows are never read).
            for pb in range(4):
                g4 = slice(32 * pb, 32 * pb + 4)
                nc.vector.tensor_copy(wtr[g4, :], wt[g4, :])
                nc.scalar.copy(rtr[g4, :], rt[g4, :])

        ppool = ctx.enter_context(tc.tile_pool(name="psum", bufs=2, space="PSUM"))

        for pb in range(4):
            bp = 32 * pb
            for t in range(NT):
                lhsT = wtr[bp : bp + 4, t * 128 : (t + 1) * 128]
                col = pb * NT + t
                for h in range(2):
                    ph = ppool.tile([128, HALF], f32, tag="ps", name="ph")
                    for j in range(4):
                        nc.tensor.matmul(
                            ph[:, j * 512 : (j + 1) * 512],
                            lhsT,
                            rtr[
                                bp : bp + 4,
                                h * HALF + j * 512 : h * HALF + (j + 1) * 512,
                            ],
                            start=True,
                            stop=True,
                            tile_position=(bp, 0),
                        )
                    nc.vector.tensor_reduce(
                        mincols[:, h * 128 + col : h * 128 + col + 1],
                        ph[:],
                        axis=mybir.AxisListType.X,
                        op=Alu.min,
                    )

        # Combine the two halves' mins, sum everything per partition, and add
        # the norm-sum column.
        nc.vector.tensor_tensor(
            fmin[:], mincols[:, 0:128], mincols[:, 128:256], op=Alu.min
        )
        nc.vector.tensor_reduce(
            r1a[:], fmin[:], axis=mybir.AxisListType.X, op=Alu.add
        )
        nc.vector.tensor_add(r1[:], r1a[:], mincols[:, 256:257])
        nc.gpsimd.dma_start(out=out_d[:], in_=r1[:])

    nc.compile()
    return nc


def _get_nc():
    if "nc" not in _CACHE:
        _CACHE["nc"] = _build()
    return _CACHE["nc"]


def _in_maps(inp, tgt, mask):
    inp = np.ascontiguousarray(inp, dtype=np.float32)
    tgt = np.ascontiguousarray(tgt, dtype=np.float32)
    mask = np.ascontiguousarray(mask, dtype=np.float32)
    return [
        {
            "inp": inp[c * BPC : (c + 1) * BPC],
            "tgt": tgt[c * BPC : (c + 1) * BPC],
            "mask": mask[c * BPC : (c + 1) * BPC],
        }
        for c in range(NCORES)
    ]


def _run(in_maps, **kwargs):
    from concourse.bass_utils import run_bass_kernel_spmd

    return run_bass_kernel_spmd(_get_nc(), in_maps, list(range(NCORES)), **kwargs)


def kernel(inp, tgt, mask):
    res = _run(_in_maps(inp, tgt, mask))
    total = 0.0
    for r in res.results:
        total += float(r["out"].sum())
    return np.float32(total / (B * N))


# revision 49
# speedup vs baseline: 1.1416x; 1.1416x over previous
"""Chamfer distance loss on Trainium2 (Bass/Tile), 8-core SPMD.

Reference math per batch b (inp/tgt: (B, C, N), mask: (B, N)):
    x = inp[b].T * mask[b,:,None]   # (N, 3)
    y = tgt[b].T * mask[b,:,None]
    d[n,m]  = ||x_n||^2 + ||y_m||^2 - 2 x_n.y_m
    loss    = mean(min_m d) + mean(min_n d)      (means over all B*N)

Device decomposition (data-parallel, 2 batches per core), using linearity
of the mean to split the norm terms out of the min:
    sum_n dist1 = sum_n min_m (y2[m] - 2 x_n.y_m) + sum_n x2[n]
    sum_m dist2 = sum_m min_n (x2[n] - 2 x_n.y_m) + sum_m y2[m]
Each min_m pass is a K=4 augmented matmul  [x0,x1,x2,1]^T . [-2y0,-2y1,-2y2,y2]
producing g[n,m] rows in PSUM, reduced with row-min only (no partition
reductions anywhere).  Row mins use the fused DVE tensor_tensor_reduce
(min elementwise of a PSUM half and an ACT-copied SBUF half, then min
along the free axis), so DVE and ACT split the reduction bandwidth.
Matmuls run as float32r (full PE rate at free dim 512, vs 4x slower fp32).

Host: shard batches across 8 cores, run SPMD, sum the per-core partial
sums and divide by B*N.
"""

import numpy as np

B, C, N = 16, 3, 4096
NCORES = 8
BPC = B // NCORES        # batches per core
NT = N // 128            # 32 n-tiles per pass
HALF = N // 2            # 2048 = one 4-bank PSUM group
BIG = float(np.finfo(np.float32).max)

_CACHE = {}


def _build():
    """Build the single-core Bass program (same program runs on all 8 cores
    with different input data)."""
    from contextlib import ExitStack

    from concourse import bacc, bass, mybir, tile  # noqa: F401

    f32 = mybir.dt.float32
    f32r = mybir.dt.float32r
    Alu = mybir.AluOpType

    nc = bacc.Bacc(trn_type="TRN2", target_bir_lowering=False, debug=False)

    inp_d = nc.dram_tensor("inp", [BPC, C, N], f32, kind="ExternalInput").ap()
    tgt_d = nc.dram_tensor("tgt", [BPC, C, N], f32, kind="ExternalInput").ap()
    mask_d = nc.dram_tensor("mask", [BPC, N], f32, kind="ExternalInput").ap()
    # Per-partition partial sums; host sums all of them and divides by B*N.
    out_d = nc.dram_tensor("out", [128, 1], f32, kind="ExternalOutput").ap()

    with tile.TileContext(nc) as tc, ExitStack() as ctx:
        pool = ctx.enter_context(tc.tile_pool(name="main", bufs=1))

        # Every compute op's partition pattern must start in row group 0
        # (offset < stride) or at 32/64/96, so operands live directly in
        # row-group-aligned mega-tiles, pass pb at partitions 32pb..32pb+3:
        #   wt group pb: rows 0-2 = -2*masked data, row 3 = 1.0   (stationary)
        #   rt group pb: rows 0-2 =    masked data, row 3 = norm  (moving)
        # Pass pairing (lhsT.T @ rhs = norm[m] - 2 v.w):
        #   pb0: -2x(b0) . y(b0)   pb1: -2y(b0) . x(b0)   pb2/pb3: batch 1
        # Final operands are float32r typed: the rounding copy below is their
        # sole writer, which satisfies the BIR verifier's "rounded to FP32r"
        # requirement for fp32r matmul inputs.
        wtr = pool.tile([128, N], f32r)
        rtr = pool.tile([128, N], f32r)
        # Columns 0..127: row-min of each (pass, n-tile)'s first PSUM half;
        # columns 128..255: second half; column 256: norm sums (rows 0-3).
        mincols = pool.tile([128, 8 * NT + 1], f32)
        fmin = pool.tile([128, 4 * NT], f32)
        r1a = pool.tile([128, 1], f32)
        r1 = pool.tile([128, 1], f32)

        # Min columns start at +BIG (neutral for the final min-combine — the
        # ACT/bf16-merged tiles only write their half-0 column); the norm-sum
        # column starts at 0 (neutral for the final add).
        nc.gpsimd.memset(mincols[:, 0 : 8 * NT], BIG)
        nc.gpsimd.memset(mincols[:, 8 * NT : 8 * NT + 1], 0.0)

        with tc.tile_pool(name="prep", bufs=1) as prpool:
            wt = prpool.tile([128, N], f32)
            rt = prpool.tile([128, N], f32)
            mr = prpool.tile([128, N], f32)  # per-batch mask rows (data rows only)
            dd = prpool.tile([4, 3 * N], f32)  # group pb's 3 masked rows, concat
            n4 = prpool.tile([4, N], f32)    # norm rows, group-major
            ones4 = prpool.tile([4, N], f32)

            nc.gpsimd.memset(ones4[:], 1.0)

            # Raw loads; group sources: wt <- (x0,y0,x1,y1), rt <- (y0,x0,y1,x1)
            # Only rows the matmuls read (32pb+0..3) are ever written/read, so
            # wt/rt need no zero-fill and every DMA below has at most 1 wait
            # (the DMA descriptor path supports only a single sync wait).
            for pb, b in enumerate((0, 0, 1, 1)):
                wsrc = inp_d[b] if pb % 2 == 0 else tgt_d[b]
                rsrc = tgt_d[b] if pb % 2 == 0 else inp_d[b]
                nc.gpsimd.dma_start(out=wt[32 * pb : 32 * pb + 3, :], in_=wsrc)
                nc.gpsimd.dma_start(out=rt[32 * pb : 32 * pb + 3, :], in_=rsrc)
                # DRAM-source broadcast AP: one DMA replicates the mask row.
                nc.gpsimd.dma_start(
                    out=mr[32 * pb : 32 * pb + 3, :],
                    in_=mask_d[b : b + 1, :].broadcast_to((3, N)),
                )

            # Mask data rows; -2 scale on wt's data rows; ones rows via DMA.
            for pb in range(4):
                g = slice(32 * pb, 32 * pb + 3)
                nc.vector.tensor_mul(rt[g, :], rt[g, :], mr[g, :])
                nc.vector.tensor_mul(wt[g, :], wt[g, :], mr[g, :])
                nc.scalar.mul(wt[g, :], wt[g, :], -2.0)
                nc.gpsimd.dma_start(
                    out=wt[32 * pb + 3 : 32 * pb + 4, :], in_=ones4[pb : pb + 1, :]
                )

            # Norm rows: gather each group's 3 masked rows into partition pb
            # (concatenated along free dim), square in place, sum the spans.
            for pb in range(4):
                nc.gpsimd.dma_start(
                    out=dd[pb : pb + 1, :], in_=rt[32 * pb : 32 * pb + 3, :]
                )
            nc.vector.tensor_mul(dd[:], dd[:], dd[:])
            nc.vector.tensor_add(n4[:], dd[:, 0:N], dd[:, N : 2 * N])
            nc.vector.tensor_add(n4[:], n4[:], dd[:, 2 * N : 3 * N])
            for pb in range(4):
                nc.gpsimd.dma_start(
                    out=rt[32 * pb + 3 : 32 * pb + 4, :], in_=n4[pb : pb + 1, :]
                )
            # sum_n x2 + sum_m y2 terms (linearity of the mean) go straight
            # into mincols' extra column.
            nc.vector.tensor_reduce(
                mincols[0:4, 8 * NT : 8 * NT + 1],
                n4[:],
                axis=mybir.AxisListType.X,
                op=Alu.add,
            )

            # Rounding copies into the fp32r operand tiles, per group (the
            # in-between rows are never read).
            for pb in range(4):
                g4 = slice(32 * pb, 32 * pb + 4)
                nc.vector.tensor_copy(wtr[g4, :], wt[g4, :])
                nc.scalar.copy(rtr[g4, :], rt[g4, :])

        ppool = ctx.enter_context(tc.tile_pool(name="psum", bufs=2, space="PSUM"))
        spool = ctx.enter_context(tc.tile_pool(name="scopy", bufs=4))
        mpool = ctx.enter_context(tc.tile_pool(name="merge", bufs=2))
        bf16 = mybir.dt.bfloat16

        # Reduction split: for 15/16 of tile-pairs, the otherwise-idle ACT
        # copy-converts both PSUM halves to bf16 SBUF and DVE merges them with
        # a 2x-mode bf16 tensor_tensor min before a single reduce (3.4us DVE
        # vs 4.5us direct); the rest reduce straight from PSUM.  Balances
        # DVE ~ ACT occupancy.
        for pb in range(4):
            bp = 32 * pb
            for t in range(NT):
                lhsT = wtr[bp : bp + 4, t * 128 : (t + 1) * 128]
                col = pb * NT + t
                halves = []
                for h in range(2):
                    ph = ppool.tile([128, HALF], f32, tag="ps", name="ph")
                    for j in range(4):
                        nc.tensor.matmul(
                            ph[:, j * 512 : (j + 1) * 512],
                            lhsT,
                            rtr[
                                bp : bp + 4,
                                h * HALF + j * 512 : h * HALF + (j + 1) * 512,
                            ],
                            start=True,
                            stop=True,
                            tile_position=(bp, 0),
                        )
                    halves.append(ph)
                if col % 16 == 0:
                    for h, ph in enumerate(halves):
                        nc.vector.tensor_reduce(
                            mincols[:, h * 128 + col : h * 128 + col + 1],
                            ph[:],
                            axis=mybir.AxisListType.X,
                            op=Alu.min,
                        )
                else:
                    s0 = spool.tile([128, HALF], bf16, tag="sc", name="s0")
                    s1 = spool.tile([128, HALF], bf16, tag="sc", name="s1")
                    nc.scalar.copy(s0[:], halves[0][:])
                    nc.scalar.copy(s1[:], halves[1][:])
                    mg = mpool.tile([128, HALF], bf16, tag="mg", name="mg")
                    nc.vector.tensor_tensor(mg[:], s0[:], s1[:], op=Alu.min)
                    nc.vector.tensor_reduce(
                        mincols[:, col : col + 1],
                        mg[:],
                        axis=mybir.AxisListType.X,
                        op=Alu.min,
                    )

        # Combine the two halves' mins, sum everything per partition, and add
        # the norm-sum column.
        nc.vector.tensor_tensor(
            fmin[:], mincols[:, 0:128], mincols[:, 128:256], op=Alu.min
        )
        nc.vector.tensor_reduce(
            r1a[:], fmin[:], axis=mybir.AxisListType.X, op=Alu.add
        )
        nc.vector.tensor_add(r1[:], r1a[:], mincols[:, 256:257])
        nc.gpsimd.dma_start(out=out_d[:], in_=r1[:])

    nc.compile()
    return nc


def _get_nc():
    if "nc" not in _CACHE:
        _CACHE["nc"] = _build()
    return _CACHE["nc"]


def _in_maps(inp, tgt, mask):
    inp = np.ascontiguousarray(inp, dtype=np.float32)
    tgt = np.ascontiguousarray(tgt, dtype=np.float32)
    mask = np.ascontiguousarray(mask, dtype=np.float32)
    return [
        {
            "inp": inp[c * BPC : (c + 1) * BPC],
            "tgt": tgt[c * BPC : (c + 1) * BPC],
            "mask": mask[c * BPC : (c + 1) * BPC],
        }
        for c in range(NCORES)
    ]


def _run(in_maps, **kwargs):
    from concourse.bass_utils import run_bass_kernel_spmd

    return run_bass_kernel_spmd(_get_nc(), in_maps, list(range(NCORES)), **kwargs)


def kernel(inp, tgt, mask):
    res = _run(_in_maps(inp, tgt, mask))
    total = 0.0
    for r in res.results:
        total += float(r["out"].sum())
    return np.float32(total / (B * N))
